# revision 1
# baseline (speedup 1.0000x reference)
"""Triplane embedding-lookup + MLP kernel for Trainium2 (8 NeuronCores), v2.

Strategy (vs v1's 96 indirect_dma_start per block):
  - Host: bucket-sort the 1M points by (band(floor_y(c1)), band(floor_y(c2)))
    into 8x8 buckets of <=16384 points (padded). c1-band -> core. Within a
    bucket, every point's patch-table row index for all 3 planes falls in a
    32768-row window -> int16 indices -> one dma_gather per plane per
    1024-point block (SWDGE embedding gather, 4 queues) instead of per-point
    descriptors. Patch table f32 (512B rows; measured same desc cost as 256B).
  - Device per block: compute window-local int16 cell indices + bilinear
    weights, 3x dma_gather, weighted-combine on DVE, 4-layer MLP on PE (bf16).
  - Host unsorts the output.
"""

import sys

sys.path.insert(0, "/opt/trn_rl_repo")

from contextlib import ExitStack

import numpy as np

RES = 512
CELLS = RES * RES
EMB = 32
HID = 128
N = 1_000_000
NCORES = 8

BROWS = 64          # plane rows per band
WIN = BROWS * RES   # 32768 rows per window (int16-addressable)
BCAP = 16384        # points per bucket (padded); max observed ~15.9k
NPB = 8             # buckets per core
NP = NPB * BCAP     # 131072 points per core
NB = 2048           # points per block (2 dma_gathers of 1024 per plane)
NBLK_B = BCAP // NB  # 16 blocks per bucket
KB = NB // 128      # 8 points per partition per block
BATCH = 4           # k-groups per MLP batch -> 512 points per matmul
NBATCH = KB // BATCH

# plane -> (x_coord_index, y_coord_index); x indexes W, y indexes H
PAIRS = ((0, 1), (1, 2), (0, 2))

LAST_RESULTS = None
_BUILT = {}


def _build_nc(npb: int = NPB, nblk_b: int = NBLK_B, do_finalize: bool = True,
              debug: bool = False, sim_cast: bool = False):
    # HW f32->i32 cast is rint -> cast(pix-0.5) == floor(pix). CoreSim
    # truncates -> cast(pix) == floor(pix). sim_cast picks the sim variant.
    from concourse import bacc, mybir
    import concourse.tile as tile
    from concourse.masks import make_identity

    dt = mybir.dt
    f32 = dt.float32
    i32 = dt.int32
    i16 = dt.int16
    bf16 = dt.bfloat16
    mult = mybir.AluOpType.mult
    add = mybir.AluOpType.add
    AF = mybir.ActivationFunctionType

    np_pts = npb * nblk_b * NB

    nc = bacc.Bacc("TRN2", target_bir_lowering=False, num_swdge_queues=4)

    pt0d = nc.dram_tensor("pt0", [WIN, 128], f32, kind="ExternalInput")
    pt1d = nc.dram_tensor("pt1", [npb * WIN, 128], f32, kind="ExternalInput")
    pt2d = nc.dram_tensor("pt2", [npb * WIN, 128], f32, kind="ExternalInput")
    ciD = nc.dram_tensor("ci", [128, (np_pts // 16) * 3], f32, kind="ExternalInput")
    cwD = nc.dram_tensor("cw", [128, (np_pts // 128) * 3], f32, kind="ExternalInput")
    c1offd = nc.dram_tensor("c1off", [128, 9], f32, kind="ExternalInput")
    w0d = nc.dram_tensor("w0t", [EMB, HID], bf16, kind="ExternalInput")
    w1d = nc.dram_tensor("w1t", [HID, HID], bf16, kind="ExternalInput")
    w2d = nc.dram_tensor("w2t", [HID, HID], bf16, kind="ExternalInput")
    w3d = nc.dram_tensor("w3t", [HID, 1], bf16, kind="ExternalInput")
    b0d = nc.dram_tensor("b0c", [HID, 1], f32, kind="ExternalInput")
    b1d = nc.dram_tensor("b1c", [HID, 1], f32, kind="ExternalInput")
    b2d = nc.dram_tensor("b2c", [HID, 1], f32, kind="ExternalInput")
    b3d = nc.dram_tensor("b3c", [1, 1], f32, kind="ExternalInput")
    outd = nc.dram_tensor("out", [np_pts], f32, kind="ExternalOutput")
    outv = outd[:].unsqueeze(0)
    nblocks = npb * nblk_b
    if debug:
        dbg_idx = nc.dram_tensor("dbg_idx", [128, nblocks * 3 * (NB // 16)], i16,
                                 kind="ExternalOutput")
        dbg_feats = nc.dram_tensor("dbg_feats", [128, nblocks * KB * EMB], bf16,
                                   kind="ExternalOutput")
        dbg_g = nc.dram_tensor("dbg_g", [128, nblocks * KB * 128], f32,
                               kind="ExternalOutput")

    with tile.TileContext(nc) as tc, ExitStack() as ctx:
        cpool = ctx.enter_context(tc.tile_pool(name="consts", bufs=1))

        def ct(shape, dtp, tag):
            return cpool.tile(shape, dtp, tag=tag, name=tag)

        w0s = ct([EMB, HID], bf16, "w0s")
        w1s = ct([HID, HID], bf16, "w1s")
        w2s = ct([HID, HID], bf16, "w2s")
        w3s = ct([HID, 1], bf16, "w3s")
        b0s = ct([HID, 1], f32, "b0s")
        b1s = ct([HID, 1], f32, "b1s")
        b2s = ct([HID, 1], f32, "b2s")
        b3s = ct([1, 1], f32, "b3s")
        c1s = ct([128, 9], f32, "c1s")
        ident = ct([128, 128], bf16, "ident")
        cws = ct([128, (np_pts // 128) * 3], f32, "cws")
        for s, d in ((w0s, w0d), (w1s, w1d), (w2s, w2d), (w3s, w3d),
                     (b0s, b0d), (b1s, b1d), (b2s, b2d), (b3s, b3d),
                     (c1s, c1offd), (cws, cwD)):
            nc.sync.dma_start(s[:], d[:])
        make_identity(nc, ident[:])

        bpool = ctx.enter_context(tc.tile_pool(name="bpool", bufs=2))
        work = ctx.enter_context(tc.tile_pool(name="work", bufs=3))
        gpool = ctx.enter_context(tc.tile_pool(name="gather", bufs=3))
        psum = ctx.enter_context(tc.tile_pool(name="psum", bufs=2, space="PSUM"))

        def bt(shape, dtp, tag, bufs=2):
            return bpool.tile(shape, dtp, tag=tag, name=tag, bufs=bufs)

        def wt(shape, dtp, tag, bufs=3):
            return work.tile(shape, dtp, tag=tag, name=tag, bufs=bufs)

        S = NB // 16

        wts_view = {}

        def emit_weights(j):
            # per-bucket bilinear weights (points l = j*128 .. j*128+127)
            cwb = cws[:, j * 384:(j + 1) * 384]
            pixw = bt([128, 384], f32, "pixw")
            nc.scalar.activation(pixw[:], cwb, AF.Copy, bias=255.5, scale=255.5)
            ciw = bt([128, 384], i32, "ciw")
            nc.scalar.activation(ciw[:], cwb, AF.Copy,
                                 bias=(255.5 if sim_cast else 255.0), scale=255.5)
            cfw = bt([128, 384], f32, "cfw")
            nc.scalar.activation(cfw[:], ciw[:], AF.Copy)
            frw = bt([128, 384], f32, "frw")
            nc.vector.tensor_sub(frw[:], pixw[:], cfw[:])
            omfw = bt([128, 384], f32, "omfw")
            nc.scalar.activation(omfw[:], frw[:], AF.Copy, bias=1.0, scale=-1.0)
            fr3 = frw[:].rearrange("p (l c) -> p l c", c=3)
            omf3 = omfw[:].rearrange("p (l c) -> p l c", c=3)

            wts = bt([128, 3 * 4 * 128], f32, "wts")
            wts4 = wts[:].rearrange("p (pl cc l) -> p pl cc l", pl=3, cc=4)
            for pl, (xc, yc) in enumerate(PAIRS):
                fx, fy = fr3[:, :, xc], fr3[:, :, yc]
                gx, gy = omf3[:, :, xc], omf3[:, :, yc]
                nc.vector.tensor_tensor(out=wts4[:, pl, 0], in0=gy, in1=gx, op=mult)
                nc.vector.tensor_tensor(out=wts4[:, pl, 1], in0=gy, in1=fx, op=mult)
                nc.vector.tensor_tensor(out=wts4[:, pl, 2], in0=fy, in1=gx, op=mult)
                nc.vector.tensor_tensor(out=wts4[:, pl, 3], in0=fy, in1=fx, op=mult)
            wts_view[j] = wts4

        def emit_idx(B):
            # index path: floors on ACT (HW rint cast), chain on DVE, Pool-free
            j = B // nblk_b
            cib = wt([128, 3 * S], f32, "cib")
            nc.sync.dma_start(cib[:], ciD[:, B * 3 * S:(B + 1) * 3 * S])
            ci32 = wt([128, 3 * S], i32, "ci32")
            nc.scalar.activation(ci32[:], cib[:], AF.Copy,
                                 bias=(255.5 if sim_cast else 255.0), scale=255.5)
            cf = wt([128, 3 * S], f32, "cf")
            nc.scalar.activation(cf[:], ci32[:], AF.Copy)
            cf3 = cf[:].rearrange("p (s c) -> p s c", c=3)

            idx16 = wt([128, 3 * S], i16, "idx16", bufs=5)
            idx163 = idx16[:].rearrange("p (pl s) -> p pl s", pl=3)
            for pl, (xc, yc) in enumerate(PAIRS):
                x0 = cf3[:, :, xc]
                y0 = cf3[:, :, yc]
                yl = wt([128, S], f32, "yl")
                bcol = 0 if pl == 0 else j + 1
                nc.scalar.activation(yl[:], y0, AF.Relu,
                                     bias=c1s[:, bcol:bcol + 1])
                ylc = wt([128, S], f32, "ylc")
                nc.vector.tensor_scalar_min(ylc[:], yl[:], float(BROWS - 1))
                idxf = wt([128, S], f32, "idxf")
                nc.vector.scalar_tensor_tensor(
                    out=idxf[:], in0=ylc[:], scalar=float(RES),
                    in1=x0, op0=mult, op1=add)
                nc.vector.tensor_copy(idx163[:, pl], idxf[:])
            return idx16, idx163

        def emit_gathers(B, idx163):
            j = B // nblk_b
            srcs = (pt0d[:],
                    pt1d[:][j * WIN:(j + 1) * WIN, :],
                    pt2d[:][j * WIN:(j + 1) * WIN, :])
            gs = []
            half = NB // 2
            for pl in range(3):
                g = gpool.tile([128, KB, 128], f32, tag=f"g{pl}",
                               name=f"g{pl}", bufs=3)
                for h in range(2):
                    nc.gpsimd.dma_gather(
                        out_ap=g[:, h * (KB // 2):(h + 1) * (KB // 2), :],
                        in_ap=srcs[pl],
                        idxs_ap=idx163[:, pl, h * (half // 16):(h + 1) * (half // 16)],
                        num_idxs=half, num_idxs_reg=half, elem_size=128,
                        queue_num=(6 * B + 2 * pl + h) % 4)
                gs.append(g)
            return gs

        def emit_tail(B, idx16, gs):
            j, b = divmod(B, nblk_b)
            if debug:
                nc.sync.dma_start(
                    dbg_idx[:, B * 3 * S:(B + 1) * 3 * S], idx16[:])
                nc.sync.dma_start(
                    dbg_g[:, B * KB * 128:(B + 1) * KB * 128],
                    gs[0][:].rearrange("p k e -> p (k e)"))

            # weighted combine -> feats [128, KB*EMB] bf16
            wts4 = wts_view[j]
            wtsb = wts4[:, :, :, b * KB:(b + 1) * KB]
            acc = wt([128, KB * EMB], f32, "acc")
            acc3 = acc[:].rearrange("p (k f) -> p k f", f=EMB)
            feats = wt([128, KB * EMB], bf16, "feats")
            terms = [(pl, cc) for pl in range(3) for cc in range(4)]
            for i, (pl, cc) in enumerate(terms):
                w_b = wtsb[:, pl, cc].unsqueeze(2).to_broadcast(
                    [128, KB, EMB])
                gsl = gs[pl][:, :, cc * EMB:(cc + 1) * EMB]
                if i == 0:
                    nc.vector.tensor_tensor(out=acc3, in0=gsl, in1=w_b,
                                            op=mult)
                    continue
                prod = wt([128, KB * EMB], f32, "prod")
                nc.vector.tensor_tensor(
                    out=prod[:].rearrange("p (k f) -> p k f", f=EMB),
                    in0=gsl, in1=w_b, op=mult)
                if i == len(terms) - 1:
                    nc.vector.tensor_add(out=feats[:], in0=acc[:],
                                         in1=prod[:])
                else:
                    nc.vector.tensor_add(out=acc[:], in0=acc[:],
                                         in1=prod[:])

            if debug:
                nc.sync.dma_start(
                    dbg_feats[:, B * KB * EMB:(B + 1) * KB * EMB], feats[:])

            # MLP
            for jj in range(NBATCH):
                ftp = psum.tile([EMB, BATCH * 128], bf16, tag="ftp",
                                name="ftp", space="PSUM", bufs=2)
                for kk in range(BATCH):
                    col = (jj * BATCH + kk) * EMB
                    nc.tensor.transpose(
                        out=ftp[:, kk * 128:(kk + 1) * 128],
                        in_=feats[:, col:col + EMB],
                        identity=ident[:])
                fts = wt([EMB, BATCH * 128], bf16, "fts")
                nc.scalar.activation(fts[:], ftp[:], AF.Copy)

                mm0 = psum.tile([HID, BATCH * 128], f32, tag="mm",
                                name="mm", space="PSUM", bufs=3)
                nc.tensor.matmul(out=mm0[:], lhsT=w0s[:], rhs=fts[:],
                                 start=True, stop=True)
                h0 = wt([HID, BATCH * 128], bf16, "h0")
                nc.scalar.activation(h0[:], mm0[:], AF.Relu,
                                     bias=b0s[:, 0:1])

                mm1 = psum.tile([HID, BATCH * 128], f32, tag="mm",
                                name="mm", space="PSUM", bufs=3)
                nc.tensor.matmul(out=mm1[:], lhsT=w1s[:], rhs=h0[:],
                                 start=True, stop=True)
                h1 = wt([HID, BATCH * 128], bf16, "h1")
                nc.scalar.activation(h1[:], mm1[:], AF.Relu,
                                     bias=b1s[:, 0:1])

                mm2 = psum.tile([HID, BATCH * 128], f32, tag="mm",
                                name="mm", space="PSUM", bufs=3)
                nc.tensor.matmul(out=mm2[:], lhsT=w2s[:], rhs=h1[:],
                                 start=True, stop=True)
                h2 = wt([HID, BATCH * 128], bf16, "h2")
                nc.scalar.activation(h2[:], mm2[:], AF.Relu,
                                     bias=b2s[:, 0:1])

                mm3 = psum.tile([1, BATCH * 128], f32, tag="mm3",
                                name="mm3", space="PSUM", bufs=2)
                nc.tensor.matmul(out=mm3[:], lhsT=w3s[:], rhs=h2[:],
                                 start=True, stop=True)
                res = wt([1, BATCH * 128], f32, "res")
                nc.scalar.activation(res[:], mm3[:], AF.Identity,
                                     bias=b3s[0:1, 0:1])
                o0 = (B * KB + jj * BATCH) * 128
                nc.sync.dma_start(outv[:, o0:o0 + BATCH * 128], res[:])

        # software pipeline: idx chain for B+1 issues ahead of B's tail so
        # the gather stream never waits on DVE combine work
        nblocks_all = npb * nblk_b
        emit_weights(0)
        pend = {0: emit_idx(0)}
        if nblocks_all > 1:
            pend[1] = emit_idx(1)
        for B in range(nblocks_all):
            if B + 2 < nblocks_all:
                if (B + 2) // nblk_b != (B + 1) // nblk_b:
                    emit_weights((B + 2) // nblk_b)
                pend[B + 2] = emit_idx(B + 2)
            idx16, idx163 = pend.pop(B)
            gs = emit_gathers(B, idx163)
            emit_tail(B, idx16, gs)

    if do_finalize:
        nc.finalize()
    return nc


def _get_nc():
    if "nc" not in _BUILT:
        _BUILT["nc"] = _build_nc()
    return _BUILT["nc"]


def _build_patch_tables(planes: np.ndarray) -> np.ndarray:
    # planes [3, 32, 512, 512] -> PT [3, 512*512, 128] f32
    p = planes.transpose(0, 2, 3, 1)  # [3, H, W, C]
    pt = np.zeros((3, RES, RES, 4, EMB), dtype=np.float32)
    pt[:, :, :, 0] = p
    pt[:, :, :-1, 1] = p[:, :, 1:]
    pt[:, :-1, :, 2] = p[:, 1:]
    pt[:, :-1, :-1, 3] = p[:, 1:, 1:]
    return np.ascontiguousarray(pt.reshape(3, CELLS, 4 * EMB))


def _y0_host(c: np.ndarray) -> np.ndarray:
    # mirror of the device floor: rint(c*255.5 + 255.0) in f32
    pm = (c.astype(np.float32) * np.float32(255.5)) + np.float32(255.0)
    return np.clip(np.rint(pm.astype(np.float32)).astype(np.int64), 0, 511)


def _prepare(inputs):
    import ml_dtypes

    coords = np.asarray(inputs["coordinates"], dtype=np.float32)
    planes = np.asarray(inputs["planes"], dtype=np.float32)
    bf = ml_dtypes.bfloat16
    n = coords.shape[0]

    pt = _build_patch_tables(planes)
    w0t = np.ascontiguousarray(inputs["w0"].T).astype(bf)
    w1t = np.ascontiguousarray(inputs["w1"].T).astype(bf)
    w2t = np.ascontiguousarray(inputs["w2"].T).astype(bf)
    w3t = np.ascontiguousarray(inputs["w3"].T).astype(bf)
    b0 = np.asarray(inputs["b0"], np.float32).reshape(HID, 1)
    b1 = np.asarray(inputs["b1"], np.float32).reshape(HID, 1)
    b2 = np.asarray(inputs["b2"], np.float32).reshape(HID, 1)
    b3 = np.asarray(inputs["b3"], np.float32).reshape(1, 1)

    # ---- bucket sort: key = (band(y0(c1)), band(y0(c2)))
    gband = _y0_host(coords[:, 1]) >> 6
    jband = _y0_host(coords[:, 2]) >> 6
    key = (gband * NPB + jband).astype(np.int64)
    order = np.argsort(key, kind="stable")
    counts = np.bincount(key, minlength=NCORES * NPB)
    assert counts.max() <= BCAP, f"bucket overflow: {counts.max()}"

    # padded sorted coords + original-id map
    sc = np.zeros((NCORES * NPB, BCAP, 3), np.float32)
    ids = np.full((NCORES * NPB, BCAP), -1, np.int64)
    # pad coords: band-interior values so indices stay in-window
    for k in range(NCORES * NPB):
        g, j = divmod(k, NPB)
        sc[k, :, 0] = 0.0
        sc[k, :, 1] = (64 * g + 32 - 255.5) / 255.5
        sc[k, :, 2] = (64 * j + 32 - 255.5) / 255.5
    starts = np.concatenate(([0], np.cumsum(counts)))
    for k in range(NCORES * NPB):
        idsk = order[starts[k]:starts[k + 1]]
        sc[k, :len(idsk)] = coords[idsk]
        ids[k, :len(idsk)] = idsk

    c1off_vals = []
    for g in range(NCORES):
        v = np.zeros((128, 9), np.float32)
        v[:, 0] = -64.0 * g
        v[:, 1:] = -64.0 * np.arange(NPB)[None, :]
        c1off_vals.append(v)

    in_maps = []
    for g in range(NCORES):
        score = sc[g * NPB:(g + 1) * NPB].reshape(NP, 3)
        cw = np.ascontiguousarray(
            score.reshape(NP // 128, 128, 3).transpose(1, 0, 2)
        ).reshape(128, (NP // 128) * 3)
        ci16 = score.reshape(NP // 16, 16, 3).transpose(1, 0, 2)  # [16, s, c]
        ci = np.ascontiguousarray(
            np.tile(ci16, (8, 1, 1))).reshape(128, (NP // 16) * 3)
        in_maps.append({
            "pt0": np.ascontiguousarray(pt[0][g * WIN:(g + 1) * WIN]),
            "pt1": pt[1], "pt2": pt[2],
            "ci": ci, "cw": cw, "c1off": c1off_vals[g],
            "w0t": w0t, "w1t": w1t, "w2t": w2t, "w3t": w3t,
            "b0c": b0, "b1c": b1, "b2c": b2, "b3c": b3,
        })
    return in_maps, ids.reshape(NCORES, NP), n


def kernel(**inputs: np.ndarray) -> np.ndarray:
    global LAST_RESULTS
    from concourse.bass_utils import run_bass_kernel_spmd

    in_maps, flat_ids, n = _prepare(inputs)
    nc = _get_nc()
    LAST_RESULTS = run_bass_kernel_spmd(nc, in_maps, list(range(NCORES)))

    full = np.zeros(n, np.float32)
    for g in range(NCORES):
        o = np.asarray(LAST_RESULTS.results[g]["out"], np.float32)
        m = flat_ids[g] >= 0
        full[flat_ids[g][m]] = o[m]
    return full.reshape(1, n, 1).astype(np.float32)



# revision 4
# speedup vs baseline: 1.2050x; 1.2050x over previous
"""Triplane embedding-lookup + MLP kernel for Trainium2 (8 NeuronCores), v2.

Strategy (vs v1's 96 indirect_dma_start per block):
  - Host: bucket-sort the 1M points by (band(floor_y(c1)), band(floor_y(c2)))
    into 8x8 buckets of <=16384 points (padded). c1-band -> core. Within a
    bucket, every point's patch-table row index for all 3 planes falls in a
    32768-row window -> int16 indices -> one dma_gather per plane per
    1024-point block (SWDGE embedding gather, 4 queues) instead of per-point
    descriptors. Patch table f32 (512B rows; measured same desc cost as 256B).
  - Device per block: compute window-local int16 cell indices + bilinear
    weights, 3x dma_gather, weighted-combine on DVE, 4-layer MLP on PE (bf16).
  - Host unsorts the output.
"""

import sys

sys.path.insert(0, "/opt/trn_rl_repo")

from contextlib import ExitStack

import numpy as np

RES = 512
CELLS = RES * RES
EMB = 32
HID = 128
N = 1_000_000
NCORES = 8

BROWS = 64          # plane rows per band
WIN = BROWS * RES   # 32768 rows per window (int16-addressable)
BCAP = 16384        # points per bucket (padded); max observed ~15.9k
NPB = 8             # buckets per core
NP = NPB * BCAP     # 131072 points per core
NB = 2048           # points per block (2 dma_gathers of 1024 per plane)
NBLK_B = BCAP // NB  # 16 blocks per bucket
KB = NB // 128      # 8 points per partition per block
BATCH = 4           # k-groups per MLP batch -> 512 points per matmul
NBATCH = KB // BATCH

# plane -> (x_coord_index, y_coord_index); x indexes W, y indexes H
PAIRS = ((0, 1), (1, 2), (0, 2))

LAST_RESULTS = None
_BUILT = {}


def _build_nc(npb: int = NPB, nblk_b: int = NBLK_B, do_finalize: bool = True,
              debug: bool = False, sim_cast: bool = False):
    # HW f32->i32 cast is rint -> cast(pix-0.5) == floor(pix). CoreSim
    # truncates -> cast(pix) == floor(pix). sim_cast picks the sim variant.
    from concourse import bacc, mybir
    import concourse.tile as tile
    from concourse.masks import make_identity

    dt = mybir.dt
    f32 = dt.float32
    i32 = dt.int32
    i16 = dt.int16
    bf16 = dt.bfloat16
    mult = mybir.AluOpType.mult
    add = mybir.AluOpType.add
    AF = mybir.ActivationFunctionType

    np_pts = npb * nblk_b * NB

    nc = bacc.Bacc("TRN2", target_bir_lowering=False, num_swdge_queues=4)

    pt0d = nc.dram_tensor("pt0", [WIN, 128], f32, kind="ExternalInput")
    pt1d = nc.dram_tensor("pt1", [npb * WIN, 128], f32, kind="ExternalInput")
    pt2d = nc.dram_tensor("pt2", [npb * WIN, 128], f32, kind="ExternalInput")
    ciD = nc.dram_tensor("ci", [128, (np_pts // 16) * 3], f32, kind="ExternalInput")
    cwD = nc.dram_tensor("cw", [128, (np_pts // 128) * 3], f32, kind="ExternalInput")
    c1offd = nc.dram_tensor("c1off", [128, 9], f32, kind="ExternalInput")
    w0d = nc.dram_tensor("w0t", [EMB, HID], bf16, kind="ExternalInput")
    w1d = nc.dram_tensor("w1t", [HID, HID], bf16, kind="ExternalInput")
    w2d = nc.dram_tensor("w2t", [HID, HID], bf16, kind="ExternalInput")
    w3d = nc.dram_tensor("w3t", [HID, 1], bf16, kind="ExternalInput")
    b0d = nc.dram_tensor("b0c", [HID, 1], f32, kind="ExternalInput")
    b1d = nc.dram_tensor("b1c", [HID, 1], f32, kind="ExternalInput")
    b2d = nc.dram_tensor("b2c", [HID, 1], f32, kind="ExternalInput")
    b3d = nc.dram_tensor("b3c", [1, 1], f32, kind="ExternalInput")
    outd = nc.dram_tensor("out", [np_pts], f32, kind="ExternalOutput")
    outv = outd[:].unsqueeze(0)
    nblocks = npb * nblk_b
    if debug:
        dbg_idx = nc.dram_tensor("dbg_idx", [128, nblocks * 3 * (NB // 16)], i16,
                                 kind="ExternalOutput")
        dbg_feats = nc.dram_tensor("dbg_feats", [128, nblocks * KB * EMB], bf16,
                                   kind="ExternalOutput")
        dbg_g = nc.dram_tensor("dbg_g", [128, nblocks * KB * 128], f32,
                               kind="ExternalOutput")

    with tile.TileContext(nc) as tc, ExitStack() as ctx:
        cpool = ctx.enter_context(tc.tile_pool(name="consts", bufs=1))

        def ct(shape, dtp, tag):
            return cpool.tile(shape, dtp, tag=tag, name=tag)

        w0s = ct([EMB, HID], bf16, "w0s")
        w1s = ct([HID, HID], bf16, "w1s")
        w2s = ct([HID, HID], bf16, "w2s")
        w3s = ct([HID, 1], bf16, "w3s")
        b0s = ct([HID, 1], f32, "b0s")
        b1s = ct([HID, 1], f32, "b1s")
        b2s = ct([HID, 1], f32, "b2s")
        b3s = ct([1, 1], f32, "b3s")
        c1s = ct([128, 9], f32, "c1s")
        ident = ct([128, 128], bf16, "ident")
        cws = ct([128, (np_pts // 128) * 3], f32, "cws")
        for s, d in ((w0s, w0d), (w1s, w1d), (w2s, w2d), (w3s, w3d),
                     (b0s, b0d), (b1s, b1d), (b2s, b2d), (b3s, b3d),
                     (c1s, c1offd), (cws, cwD)):
            nc.sync.dma_start(s[:], d[:])
        make_identity(nc, ident[:])

        bpool = ctx.enter_context(tc.tile_pool(name="bpool", bufs=2))
        work = ctx.enter_context(tc.tile_pool(name="work", bufs=3))
        gpool = ctx.enter_context(tc.tile_pool(name="gather", bufs=3))
        psum = ctx.enter_context(tc.tile_pool(name="psum", bufs=2, space="PSUM"))

        def bt(shape, dtp, tag, bufs=2):
            return bpool.tile(shape, dtp, tag=tag, name=tag, bufs=bufs)

        def wt(shape, dtp, tag, bufs=3):
            return work.tile(shape, dtp, tag=tag, name=tag, bufs=bufs)

        S = NB // 16

        wts_view = {}

        def emit_weights(j):
            # per-bucket bilinear weights (points l = j*128 .. j*128+127)
            cwb = cws[:, j * 384:(j + 1) * 384]
            pixw = bt([128, 384], f32, "pixw")
            nc.scalar.activation(pixw[:], cwb, AF.Copy, bias=255.5, scale=255.5)
            ciw = bt([128, 384], i32, "ciw")
            nc.scalar.activation(ciw[:], cwb, AF.Copy,
                                 bias=(255.5 if sim_cast else 255.0), scale=255.5)
            cfw = bt([128, 384], f32, "cfw")
            nc.scalar.activation(cfw[:], ciw[:], AF.Copy)
            frw = bt([128, 384], f32, "frw")
            nc.vector.tensor_sub(frw[:], pixw[:], cfw[:])
            omfw = bt([128, 384], f32, "omfw")
            nc.scalar.activation(omfw[:], frw[:], AF.Copy, bias=1.0, scale=-1.0)
            fr3 = frw[:].rearrange("p (l c) -> p l c", c=3)
            omf3 = omfw[:].rearrange("p (l c) -> p l c", c=3)

            wts = bt([128, 3 * 4 * 128], f32, "wts")
            wts4 = wts[:].rearrange("p (pl cc l) -> p pl cc l", pl=3, cc=4)
            for pl, (xc, yc) in enumerate(PAIRS):
                fx, fy = fr3[:, :, xc], fr3[:, :, yc]
                gx, gy = omf3[:, :, xc], omf3[:, :, yc]
                nc.vector.tensor_tensor(out=wts4[:, pl, 0], in0=gy, in1=gx, op=mult)
                nc.vector.tensor_tensor(out=wts4[:, pl, 1], in0=gy, in1=fx, op=mult)
                nc.vector.tensor_tensor(out=wts4[:, pl, 2], in0=fy, in1=gx, op=mult)
                nc.vector.tensor_tensor(out=wts4[:, pl, 3], in0=fy, in1=fx, op=mult)
            wts_view[j] = wts4

        def emit_idx(B):
            # index path: floors on ACT (HW rint cast), chain on DVE, Pool-free
            j = B // nblk_b
            cib = wt([128, 3 * S], f32, "cib")
            nc.sync.dma_start(cib[:], ciD[:, B * 3 * S:(B + 1) * 3 * S])
            ci32 = wt([128, 3 * S], i32, "ci32")
            nc.scalar.activation(ci32[:], cib[:], AF.Copy,
                                 bias=(255.5 if sim_cast else 255.0), scale=255.5)
            cf = wt([128, 3 * S], f32, "cf")
            nc.scalar.activation(cf[:], ci32[:], AF.Copy)
            cf3 = cf[:].rearrange("p (s c) -> p s c", c=3)

            idx16 = wt([128, 3 * S], i16, "idx16", bufs=5)
            idx163 = idx16[:].rearrange("p (pl s) -> p pl s", pl=3)
            for pl, (xc, yc) in enumerate(PAIRS):
                x0 = cf3[:, :, xc]
                y0 = cf3[:, :, yc]
                yl = wt([128, S], f32, "yl")
                bcol = 0 if pl == 0 else j + 1
                nc.scalar.activation(yl[:], y0, AF.Relu,
                                     bias=c1s[:, bcol:bcol + 1])
                ylc = wt([128, S], f32, "ylc")
                nc.vector.tensor_scalar_min(ylc[:], yl[:], float(BROWS - 1))
                idxf = wt([128, S], f32, "idxf")
                nc.vector.scalar_tensor_tensor(
                    out=idxf[:], in0=ylc[:], scalar=float(RES),
                    in1=x0, op0=mult, op1=add)
                nc.vector.tensor_copy(idx163[:, pl], idxf[:])
            return idx16, idx163

        def emit_gathers(B, idx163):
            j = B // nblk_b
            srcs = (pt0d[:],
                    pt1d[:][j * WIN:(j + 1) * WIN, :],
                    pt2d[:][j * WIN:(j + 1) * WIN, :])
            gs = []
            quarter = NB // 4
            for pl in range(3):
                g = gpool.tile([128, KB, 128], f32, tag=f"g{pl}",
                               name=f"g{pl}", bufs=3)
                for h in range(4):
                    nc.gpsimd.dma_gather(
                        out_ap=g[:, h * (KB // 4):(h + 1) * (KB // 4), :],
                        in_ap=srcs[pl],
                        idxs_ap=idx163[:, pl, h * (quarter // 16):(h + 1) * (quarter // 16)],
                        num_idxs=quarter, num_idxs_reg=quarter, elem_size=128,
                        queue_num=(12 * B + 4 * pl + h) % 4)
                gs.append(g)
            return gs

        def emit_tail(B, idx16, gs):
            j, b = divmod(B, nblk_b)
            if debug:
                nc.sync.dma_start(
                    dbg_idx[:, B * 3 * S:(B + 1) * 3 * S], idx16[:])
                nc.sync.dma_start(
                    dbg_g[:, B * KB * 128:(B + 1) * KB * 128],
                    gs[0][:].rearrange("p k e -> p (k e)"))

            # weighted combine -> feats [128, KB*EMB] bf16
            wts4 = wts_view[j]
            wtsb = wts4[:, :, :, b * KB:(b + 1) * KB]
            acc = wt([128, KB * EMB], f32, "acc")
            acc3 = acc[:].rearrange("p (k f) -> p k f", f=EMB)
            feats = wt([128, KB * EMB], bf16, "feats")
            terms = [(pl, cc) for pl in range(3) for cc in range(4)]
            for i, (pl, cc) in enumerate(terms):
                w_b = wtsb[:, pl, cc].unsqueeze(2).to_broadcast(
                    [128, KB, EMB])
                gsl = gs[pl][:, :, cc * EMB:(cc + 1) * EMB]
                if i == 0:
                    nc.vector.tensor_tensor(out=acc3, in0=gsl, in1=w_b,
                                            op=mult)
                    continue
                prod = wt([128, KB * EMB], f32, "prod")
                nc.vector.tensor_tensor(
                    out=prod[:].rearrange("p (k f) -> p k f", f=EMB),
                    in0=gsl, in1=w_b, op=mult)
                if i == len(terms) - 1:
                    nc.vector.tensor_add(out=feats[:], in0=acc[:],
                                         in1=prod[:])
                else:
                    nc.vector.tensor_add(out=acc[:], in0=acc[:],
                                         in1=prod[:])

            if debug:
                nc.sync.dma_start(
                    dbg_feats[:, B * KB * EMB:(B + 1) * KB * EMB], feats[:])

            # MLP
            for jj in range(NBATCH):
                ftp = psum.tile([EMB, BATCH * 128], bf16, tag="ftp",
                                name="ftp", space="PSUM", bufs=2)
                for kk in range(BATCH):
                    col = (jj * BATCH + kk) * EMB
                    nc.tensor.transpose(
                        out=ftp[:, kk * 128:(kk + 1) * 128],
                        in_=feats[:, col:col + EMB],
                        identity=ident[:])
                fts = wt([EMB, BATCH * 128], bf16, "fts")
                nc.scalar.activation(fts[:], ftp[:], AF.Copy)

                mm0 = psum.tile([HID, BATCH * 128], f32, tag="mm",
                                name="mm", space="PSUM", bufs=3)
                nc.tensor.matmul(out=mm0[:], lhsT=w0s[:], rhs=fts[:],
                                 start=True, stop=True)
                h0 = wt([HID, BATCH * 128], bf16, "h0")
                nc.scalar.activation(h0[:], mm0[:], AF.Relu,
                                     bias=b0s[:, 0:1])

                mm1 = psum.tile([HID, BATCH * 128], f32, tag="mm",
                                name="mm", space="PSUM", bufs=3)
                nc.tensor.matmul(out=mm1[:], lhsT=w1s[:], rhs=h0[:],
                                 start=True, stop=True)
                h1 = wt([HID, BATCH * 128], bf16, "h1")
                nc.scalar.activation(h1[:], mm1[:], AF.Relu,
                                     bias=b1s[:, 0:1])

                mm2 = psum.tile([HID, BATCH * 128], f32, tag="mm",
                                name="mm", space="PSUM", bufs=3)
                nc.tensor.matmul(out=mm2[:], lhsT=w2s[:], rhs=h1[:],
                                 start=True, stop=True)
                h2 = wt([HID, BATCH * 128], bf16, "h2")
                nc.scalar.activation(h2[:], mm2[:], AF.Relu,
                                     bias=b2s[:, 0:1])

                mm3 = psum.tile([1, BATCH * 128], f32, tag="mm3",
                                name="mm3", space="PSUM", bufs=2)
                nc.tensor.matmul(out=mm3[:], lhsT=w3s[:], rhs=h2[:],
                                 start=True, stop=True)
                res = wt([1, BATCH * 128], f32, "res")
                nc.scalar.activation(res[:], mm3[:], AF.Identity,
                                     bias=b3s[0:1, 0:1])
                o0 = (B * KB + jj * BATCH) * 128
                nc.sync.dma_start(outv[:, o0:o0 + BATCH * 128], res[:])

        # software pipeline: idx chain for B+1 issues ahead of B's tail so
        # the gather stream never waits on DVE combine work
        nblocks_all = npb * nblk_b
        emit_weights(0)
        pend = {0: emit_idx(0)}
        if nblocks_all > 1:
            pend[1] = emit_idx(1)
        for B in range(nblocks_all):
            if B + 2 < nblocks_all:
                if (B + 2) // nblk_b != (B + 1) // nblk_b:
                    emit_weights((B + 2) // nblk_b)
                pend[B + 2] = emit_idx(B + 2)
            idx16, idx163 = pend.pop(B)
            gs = emit_gathers(B, idx163)
            emit_tail(B, idx16, gs)

    if do_finalize:
        nc.finalize()
    return nc


def _get_nc():
    if "nc" not in _BUILT:
        _BUILT["nc"] = _build_nc()
    return _BUILT["nc"]


def _build_patch_tables(planes: np.ndarray) -> np.ndarray:
    # planes [3, 32, 512, 512] -> PT [3, 512*512, 128] f32
    p = planes.transpose(0, 2, 3, 1)  # [3, H, W, C]
    pt = np.zeros((3, RES, RES, 4, EMB), dtype=np.float32)
    pt[:, :, :, 0] = p
    pt[:, :, :-1, 1] = p[:, :, 1:]
    pt[:, :-1, :, 2] = p[:, 1:]
    pt[:, :-1, :-1, 3] = p[:, 1:, 1:]
    return np.ascontiguousarray(pt.reshape(3, CELLS, 4 * EMB))


def _y0_host(c: np.ndarray) -> np.ndarray:
    # mirror of the device floor: rint(c*255.5 + 255.0) in f32
    pm = (c.astype(np.float32) * np.float32(255.5)) + np.float32(255.0)
    return np.clip(np.rint(pm.astype(np.float32)).astype(np.int64), 0, 511)


def _prepare(inputs):
    import ml_dtypes

    coords = np.asarray(inputs["coordinates"], dtype=np.float32)
    planes = np.asarray(inputs["planes"], dtype=np.float32)
    bf = ml_dtypes.bfloat16
    n = coords.shape[0]

    pt = _build_patch_tables(planes)
    w0t = np.ascontiguousarray(inputs["w0"].T).astype(bf)
    w1t = np.ascontiguousarray(inputs["w1"].T).astype(bf)
    w2t = np.ascontiguousarray(inputs["w2"].T).astype(bf)
    w3t = np.ascontiguousarray(inputs["w3"].T).astype(bf)
    b0 = np.asarray(inputs["b0"], np.float32).reshape(HID, 1)
    b1 = np.asarray(inputs["b1"], np.float32).reshape(HID, 1)
    b2 = np.asarray(inputs["b2"], np.float32).reshape(HID, 1)
    b3 = np.asarray(inputs["b3"], np.float32).reshape(1, 1)

    # ---- bucket sort: key = (band(y0(c1)), band(y0(c2)))
    gband = _y0_host(coords[:, 1]) >> 6
    jband = _y0_host(coords[:, 2]) >> 6
    key = (gband * NPB + jband).astype(np.int64)
    order = np.argsort(key, kind="stable")
    counts = np.bincount(key, minlength=NCORES * NPB)
    assert counts.max() <= BCAP, f"bucket overflow: {counts.max()}"

    # padded sorted coords + original-id map
    sc = np.zeros((NCORES * NPB, BCAP, 3), np.float32)
    ids = np.full((NCORES * NPB, BCAP), -1, np.int64)
    # pad coords: band-interior values so indices stay in-window
    for k in range(NCORES * NPB):
        g, j = divmod(k, NPB)
        sc[k, :, 0] = 0.0
        sc[k, :, 1] = (64 * g + 32 - 255.5) / 255.5
        sc[k, :, 2] = (64 * j + 32 - 255.5) / 255.5
    starts = np.concatenate(([0], np.cumsum(counts)))
    for k in range(NCORES * NPB):
        idsk = order[starts[k]:starts[k + 1]]
        sc[k, :len(idsk)] = coords[idsk]
        ids[k, :len(idsk)] = idsk

    c1off_vals = []
    for g in range(NCORES):
        v = np.zeros((128, 9), np.float32)
        v[:, 0] = -64.0 * g
        v[:, 1:] = -64.0 * np.arange(NPB)[None, :]
        c1off_vals.append(v)

    in_maps = []
    for g in range(NCORES):
        score = sc[g * NPB:(g + 1) * NPB].reshape(NP, 3)
        cw = np.ascontiguousarray(
            score.reshape(NP // 128, 128, 3).transpose(1, 0, 2)
        ).reshape(128, (NP // 128) * 3)
        ci16 = score.reshape(NP // 16, 16, 3).transpose(1, 0, 2)  # [16, s, c]
        ci = np.ascontiguousarray(
            np.tile(ci16, (8, 1, 1))).reshape(128, (NP // 16) * 3)
        in_maps.append({
            "pt0": np.ascontiguousarray(pt[0][g * WIN:(g + 1) * WIN]),
            "pt1": pt[1], "pt2": pt[2],
            "ci": ci, "cw": cw, "c1off": c1off_vals[g],
            "w0t": w0t, "w1t": w1t, "w2t": w2t, "w3t": w3t,
            "b0c": b0, "b1c": b1, "b2c": b2, "b3c": b3,
        })
    return in_maps, ids.reshape(NCORES, NP), n


def kernel(**inputs: np.ndarray) -> np.ndarray:
    global LAST_RESULTS
    from concourse.bass_utils import run_bass_kernel_spmd

    in_maps, flat_ids, n = _prepare(inputs)
    nc = _get_nc()
    LAST_RESULTS = run_bass_kernel_spmd(nc, in_maps, list(range(NCORES)))

    full = np.zeros(n, np.float32)
    for g in range(NCORES):
        o = np.asarray(LAST_RESULTS.results[g]["out"], np.float32)
        m = flat_ids[g] >= 0
        full[flat_ids[g][m]] = o[m]
    return full.reshape(1, n, 1).astype(np.float32)

